# revision 27
# baseline (speedup 1.0000x reference)
"""Trainium2 Bass kernel for nn_ContrastiveLoss_76501957477132.

Math (see module docstring history): with T=0.3, n=512 tracks, Q=8,
M=8192, D=128: per track i the loss needs per-track sums of
exp(x@x.T/T) (all pairs) and exp(x@yf.T/T) (all y cols), plus positive
terms computed exactly on host in float64.

v2 architecture (per core; 8 cores via band pairing k / 15-k):
  Inputs are fp8e4 (quantization error averages out in the 100K-element
  denominator sums; validated 3.6e-5 end-to-end).  Work unit = one
  [128 x 512] matmul tile; 33 units per group x 4 groups = 132 units.
  PSUM: ring of 3 slots x [128, 2, 512] f32 (6 banks) + 1 colsum
  accumulation bank.  Two consumer streams alternate on the ring:
   - ACT slots (8/group = off pairs + (off14, diagA)): ScalarE exp ->
     fp8e4 strip (feeds DoubleRow ones-matmul colsums for the E_xx
     mirror) with the rowsum free via accum_out.
   - DVE slots (34 flat = 4 diagB + 64 transposed-xy units): Schraudolph
     bit-trick exp straight to fp8e4 BITS: u8 = rne(z*S8 + B8) IS the
     fp8e4 encoding of exp(z/T).  These tiles are reduced purely by
     DoubleRow ones-matmul colsums (xy is computed transposed,
     [128 y x 512 x], so the per-x sums ARE colsums), so DVE does a
     single 1x PSUM pass and needs no second reduction pass anywhere.
  Colsum matmuls are emitted in deferred batches of 6 so the ones
  weights stay loaded and the PE never stalls on a strip.  Host folds
  rowsum/colsum partials by row residue (mod 512) and finishes the
  log/mean in float64.
"""

import numpy as np
import ml_dtypes

M, D, N_TRACKS, Q = 8192, 128, 512, 8
NQ = N_TRACKS * Q  # 4096
TEMP = 0.3
N_CORES = 8
N_BANDS = M // N_TRACKS          # 16 bands of 512 rows
GROUPS = 4                       # 128-row groups per band
OFF_UNITS = 15                   # off-diagonal xx units per group
N_RHS = 25                       # 15 off + diagA + diagB + 8 yf blocks
A_SLOTS = 8                      # ACT slots per group (2 units each)
D_SLOTS = 34                     # DVE slots per core (2 units each, flat)
T_TILES = 32                     # transposed-xy y-tiles per band (all of xy)
N_ACC = A_SLOTS                  # rowsum slots per group

# Schraudolph constants: fp8e4 bits of exp(z/TEMP)
SCH8_S = 8.0 / (TEMP * np.log(2.0))
SCH8_B = 7.0 * 8.0 - 0.45

_CACHED = {}


def _build_module():
    import concourse.bacc as bacc
    import concourse.tile as tile
    import concourse.mybir as mybir

    nc = bacc.Bacc(None, target_bir_lowering=False)
    bf16 = mybir.dt.bfloat16
    f32 = mybir.dt.float32
    f8 = mybir.dt.float8e4
    u8 = mybir.dt.uint8
    ALU = mybir.AluOpType

    lhsT_d = nc.dram_tensor("lhsT", [128, GROUPS, 2, 128], f8, kind="ExternalInput")
    lhsT_off_d = nc.dram_tensor(
        "lhsT_off", [128, GROUPS, OFF_UNITS, 128], f8, kind="ExternalInput"
    )
    rhs_d = nc.dram_tensor("rhs", [128, N_RHS, 512], f8, kind="ExternalInput")
    acc_d = nc.dram_tensor("acc", [128, GROUPS, N_ACC], f32, kind="ExternalOutput")
    cs_d = nc.dram_tensor("cs", [1, 512], f32, kind="ExternalOutput")

    # ACT slot tables per group: (stat, rhs_idx); stat: 0=A 1=B 2+u=off u
    a_slots = []
    for j in range(7):
        a_slots.append(((2 + 2 * j, 2 * j), (2 + 2 * j + 1, 2 * j + 1)))
    a_slots.append(((2 + 14, 14), (0, 15)))          # (off14, diagA)
    assert len(a_slots) == A_SLOTS
    # DVE slot table, flat over the core: colsum-only tiles.
    # ("diag", s): diagB of group s.  ("T", band_r, t): transposed xy,
    # stationary = y-tile t (128 rows of yf), moving = x band block.
    d_units = [("diag", s) for s in range(GROUPS)]
    d_units += [("T", 16, t) for t in range(T_TILES)]
    d_units += [("T", 15, t) for t in range(T_TILES)]
    d_slots = [tuple(d_units[2 * i : 2 * i + 2]) for i in range(D_SLOTS)]
    assert len(d_units) == 2 * D_SLOTS

    with tile.TileContext(nc) as tc:
        with (
            tc.tile_pool(name="consts", bufs=1) as consts,
            tc.tile_pool(name="accp", bufs=1) as accp,
            tc.tile_pool(name="ostrip", bufs=2) as ostrip_pool,
            tc.tile_pool(name="dstrip", bufs=12) as dstrip_pool,
            tc.tile_pool(name="psum", bufs=3, space="PSUM") as psum_pool,
            tc.tile_pool(name="cspsum", bufs=1, space="PSUM") as cs_pool,
        ):
            # ---- warmups: ACT exp table load + PE dummy matmuls ----
            warm = consts.tile([128, 1], f32, tag="warm")
            nc.vector.memset(warm[:], 0.0)
            warm_o = consts.tile([128, 1], bf16, tag="warm_o")
            nc.scalar.activation(
                out=warm_o[:], in_=warm[:],
                func=mybir.ActivationFunctionType.Exp, scale=1.0 / TEMP,
            )
            ones8 = consts.tile([128, 2, 16], f8, tag="ones8")
            nc.vector.memset(ones8[:], 1.0)
            # small PE warm-up burst sized to finish right as the first
            # inputs land: flips the HAM clock gate (1.2 -> 2.4 GHz) before
            # real matmuls start, without delaying them
            warm_src = consts.tile([128, 512], f8, tag="warm_src")
            nc.vector.memset(warm_src[:], 0.03125)
            warm_ps = psum_pool.tile([128, 2, 512], f32, name="ring")
            for _ in range(5):
                nc.tensor.matmul(
                    warm_ps[:, 0, :], warm_src[:, 0:128], warm_src[:],
                    start=True, stop=True, skip_group_check=True,
                )

            # ---- input DMAs, roughly in first-consumption order ----
            lhsT = consts.tile([128, GROUPS, 2, 128], f8, tag="lhsT")
            lhsT_off = consts.tile([128, GROUPS, OFF_UNITS, 128], f8, tag="lhsT_off")
            rhs = consts.tile([128, N_RHS, 512], f8, tag="rhs")
            # D-slots are front-loaded in the schedule and need only lhsT +
            # rhs[16:18]; land those first (split across rings/engines), then
            # the first A-slot inputs, then the rest of the stream.
            nc.sync.dma_start(lhsT[0:64], lhsT_d[0:64])
            nc.scalar.dma_start(lhsT[64:128], lhsT_d[64:128])
            nc.sync.dma_start(rhs[0:64, 16:18], rhs_d[0:64, 16:18])
            nc.scalar.dma_start(rhs[64:128, 16:18], rhs_d[64:128, 16:18])
            nc.sync.dma_start(rhs[0:64, 0:2], rhs_d[0:64, 0:2])
            nc.scalar.dma_start(rhs[64:128, 0:2], rhs_d[64:128, 0:2])
            nc.sync.dma_start(lhsT_off[0:64, 0], lhsT_off_d[0:64, 0])
            nc.scalar.dma_start(lhsT_off[64:128, 0], lhsT_off_d[64:128, 0])
            nc.sync.dma_start(rhs[:, 18:21], rhs_d[:, 18:21])
            nc.scalar.dma_start(rhs[:, 2:4], rhs_d[:, 2:4])
            nc.sync.dma_start(rhs[:, 4:6], rhs_d[:, 4:6])
            nc.scalar.dma_start(rhs[:, 15:16], rhs_d[:, 15:16])
            nc.sync.dma_start(rhs[:, 6:8], rhs_d[:, 6:8])
            nc.scalar.dma_start(rhs[:, 21:25], rhs_d[:, 21:25])
            nc.sync.dma_start(rhs[:, 8:12], rhs_d[:, 8:12])
            nc.sync.dma_start(rhs[:, 12:15], rhs_d[:, 12:15])
            nc.sync.dma_start(lhsT_off[:, 1], lhsT_off_d[:, 1])
            nc.sync.dma_start(lhsT_off[:, 2], lhsT_off_d[:, 2])
            nc.sync.dma_start(lhsT_off[:, 3], lhsT_off_d[:, 3])

            acc_sb = accp.tile([128, GROUPS, N_ACC], f32)
            cs_ps = cs_pool.tile([1, 512], f32, name="cs_ps")

            n_cs = GROUPS * 8 + D_SLOTS  # per-group off 7 pairs+1 single; D pairs
            cs_done = 0
            pending = []  # deferred colsum matmuls: (rhs_ap, n_tiles)

            def flush_pending(k=2):
                nonlocal cs_done
                for ap, n in pending[:k]:
                    cs_done += 1
                    if n == 2:
                        nc.tensor.matmul(
                            cs_ps[:], ones8[:, :, 0:1], ap,
                            start=(cs_done == 1), stop=(cs_done == n_cs),
                            perf_mode=mybir.MatmulPerfMode.DoubleRow,
                            skip_group_check=True,
                        )
                    else:
                        nc.tensor.matmul(
                            cs_ps[:], ones8[:, 0, 0:1], ap,
                            start=(cs_done == 1), stop=(cs_done == n_cs),
                            skip_group_check=True,
                        )
                del pending[:k]

            def stat_ap(s, code):
                if code >= 2:
                    return lhsT_off[:, s, code - 2, :]
                return lhsT[:, s, code, :]

            # interleave the 4x9 ACT slots with the 30 DVE slots (Bresenham)
            sched = [("D", i) for i in range(4)]  # ramp filler: tiny inputs
            na, nd = GROUPS * A_SLOTS, D_SLOTS - 4
            ia = err = 0
            id_ = 4
            for _ in range(na + nd):
                if (err >= 0 and ia < na) or id_ >= D_SLOTS:
                    sched.append(("A", ia)); ia += 1; err -= nd
                else:
                    sched.append(("D", id_)); id_ += 1; err += na
            ostrips = {}
            for kind, idx in sched:
                ps = psum_pool.tile([128, 2, 512], f32, name="ring")
                if kind == "A":
                    s, j = divmod(idx, A_SLOTS)
                    if j == 0:
                        ostrips[s] = ostrip_pool.tile(
                            [128, A_SLOTS, 2, 512], f8, name=f"ostrip{s % 2}"
                        )
                    ostrip = ostrips[s]
                    for e, (code, r) in enumerate(a_slots[j]):
                        nc.tensor.matmul(
                            ps[:, e, :], stat_ap(s, code), rhs[:, r, :],
                            start=True, stop=True,
                        )
                    nc.scalar.activation(
                        out=ostrip[:, j], in_=ps[:],
                        func=mybir.ActivationFunctionType.Exp,
                        scale=1.0 / TEMP,
                        accum_out=acc_sb[:, s, j : j + 1],
                    )
                    if j < 7:
                        pending.append((ostrip[:, j], 2))
                    else:
                        pending.append((ostrip[:, j, 0], 1))
                        # [128, 8] f32 = 128 one-line descriptors: ~5us on a
                        # single ring.  Split by partitions so the final
                        # group's results drain in ~1.5us instead.
                        if s < GROUPS - 1:
                            nc.sync.dma_start(acc_d[0:64, s], acc_sb[0:64, s])
                            nc.scalar.dma_start(acc_d[64:128, s], acc_sb[64:128, s])
                        else:
                            for q in range(4):
                                eng = nc.sync if q % 2 == 0 else nc.scalar
                                eng.dma_start(
                                    acc_d[32 * q : 32 * q + 32, s],
                                    acc_sb[32 * q : 32 * q + 32, s],
                                )
                else:
                    for e, unit in enumerate(d_slots[idx]):
                        if unit[0] == "diag":
                            st, mv = lhsT[:, unit[1], 1, :], rhs[:, 16, :]
                        else:
                            _, band_r, t = unit
                            o = 128 * (t % 4)
                            st = rhs[:, 17 + t // 4, o : o + 128]
                            mv = rhs[:, band_r, :]
                        nc.tensor.matmul(ps[:, e, :], st, mv, start=True, stop=True)
                    dstrip = dstrip_pool.tile([128, 2, 512], u8, name="dstrip")
                    nc.vector.tensor_scalar(
                        dstrip[:], ps[:], SCH8_S, SCH8_B, ALU.mult, ALU.add
                    )
                    pending.append((dstrip[:].bitcast(f8), 2))
                if len(pending) >= 10:
                    flush_pending(10)
            flush_pending(len(pending))
            cs_sb = accp.tile([1, 512], f32, tag="cs_sb")
            nc.vector.tensor_copy(cs_sb[:], cs_ps[:])
            nc.sync.dma_start(cs_d[:], cs_sb[:])
    nc.compile()
    return nc


def _get_module():
    if "nc" not in _CACHED:
        _CACHED["nc"] = _build_module()
    return _CACHED["nc"]


def _positive_terms(x64, yf64):
    """num_xy, G, diag_self as float64 [512] vectors (exact math)."""
    xs = x64.reshape(N_BANDS, N_TRACKS, D)
    yfs = yf64.reshape(NQ // N_TRACKS, N_TRACKS, D)
    dxx = np.einsum("rid,cid->rci", xs, xs)
    dxy = np.einsum("rid,qid->rqi", xs, yfs)
    G = np.exp(dxx / TEMP).sum(axis=(0, 1))
    diag_self = np.exp(np.einsum("rid,rid->ri", xs, xs) / TEMP).sum(axis=0)
    num_xy = np.exp(dxy / TEMP).sum(axis=(0, 1))
    return num_xy, G, diag_self


def _finish(rs_seg, num_xy, G, diag_self):
    num = num_xy + (G - diag_self) / 2.0
    den = rs_seg - num_xy - G
    loss = np.mean(-np.log(num / (den + num))) / Q
    return np.asarray(loss, dtype=np.float32)


def _numpy_fallback(x, track_idxs, y):
    """Exact general-track reference in numpy (safety net only)."""
    x64 = x.astype(np.float64)
    yf64 = y.reshape(NQ, D).astype(np.float64)
    t = track_idxs.astype(np.int64)
    y_idxs = np.tile(np.arange(N_TRACKS, dtype=np.int64), Q)
    E_xy = np.exp(x64 @ yf64.T / TEMP)
    Sx = np.zeros((N_TRACKS, NQ))
    np.add.at(Sx, t, E_xy)
    Py = (y_idxs[:, None] == np.arange(N_TRACKS)[None, :]).astype(np.float64)
    num_xy = np.einsum("ik,ki->i", Sx, Py)
    den_xy = Sx.sum(axis=1) - num_xy
    E_xx = np.exp(x64 @ x64.T / TEMP)
    Sxx = np.zeros((N_TRACKS, M))
    np.add.at(Sxx, t, E_xx)
    Px = (t[:, None] == np.arange(N_TRACKS)[None, :]).astype(np.float64)
    G_diag = np.einsum("im,mi->i", Sxx, Px)
    diag_self = np.zeros(N_TRACKS)
    np.add.at(diag_self, t, np.diagonal(E_xx))
    num_xx = (G_diag - diag_self) / 2.0
    den_xx = Sxx.sum(axis=1) - G_diag
    num = num_xy + num_xx
    den = den_xy + den_xx
    loss = np.mean(-np.log(num / (den + num))) / Q
    return np.asarray(loss, dtype=np.float32)


def kernel(x, track_idxs, y):
    x = np.asarray(x, dtype=np.float32)
    y = np.asarray(y, dtype=np.float32)
    track_idxs = np.asarray(track_idxs)

    expected_tracks = np.arange(M, dtype=np.int64) % N_TRACKS
    if (
        x.shape != (M, D)
        or y.shape != (N_TRACKS, Q, D)
        or not np.array_equal(track_idxs.astype(np.int64), expected_tracks)
    ):
        return _numpy_fallback(x, track_idxs, y)

    from concourse.bass_utils import run_bass_kernel_spmd

    f8np = ml_dtypes.float8_e4m3
    yf = np.ascontiguousarray(y.reshape(NQ, D))
    xT = np.ascontiguousarray(x.T).astype(f8np)     # [128, 8192]
    yfT = np.ascontiguousarray(yf.T).astype(f8np)   # [128, 4096]
    xT_blocks = xT.reshape(128, N_BANDS, 512)
    yfT_blocks = yfT.reshape(128, Q, 512)

    def subtile(band, s):
        t = 4 * band + s
        return xT[:, 128 * t : 128 * (t + 1)]

    in_maps = []
    for k in range(N_CORES):
        A, B = k, (N_BANDS - 1) - k
        off_cols = list(range(A + 1, N_BANDS)) + list(range(B + 1, N_BANDS))
        assert len(off_cols) == OFF_UNITS
        rhs = np.empty((128, N_RHS, 512), dtype=f8np)
        for u, c in enumerate(off_cols):
            rhs[:, u] = xT_blocks[:, c]
        rhs[:, 15] = xT_blocks[:, A]
        rhs[:, 16] = xT_blocks[:, B]
        for q in range(Q):
            rhs[:, 17 + q] = yfT_blocks[:, q]

        lhsT = np.empty((128, GROUPS, 2, 128), dtype=f8np)
        lhsT_off = np.empty((128, GROUPS, OFF_UNITS, 128), dtype=f8np)
        for s in range(GROUPS):
            lhsT[:, s, 0, :] = subtile(A, s)
            lhsT[:, s, 1, :] = subtile(B, s)
            for u in range(OFF_UNITS):
                band = A if u < (N_BANDS - 1) - k else B
                lhsT_off[:, s, u, :] = subtile(band, s)
        in_maps.append(
            {
                "lhsT": np.ascontiguousarray(lhsT),
                "lhsT_off": np.ascontiguousarray(lhsT_off),
                "rhs": np.ascontiguousarray(rhs),
            }
        )

    nc = _get_module()
    res = run_bass_kernel_spmd(nc, in_maps, core_ids=list(range(N_CORES)))
    _CACHED["last_res"] = res

    # Fold partials by row residue (mod 512): group s covers residues
    # 128s + p; colsums fold by in-block column position directly.
    rs_seg = np.zeros(N_TRACKS, dtype=np.float64)
    for k in range(N_CORES):
        acc = np.asarray(res.results[k]["acc"], dtype=np.float64)  # [128, 4, 11]
        per_group = acc.sum(axis=2)                                # [128, 4]
        rs_seg += per_group.T.reshape(N_TRACKS)                    # i = 128*s + p
        rs_seg += np.asarray(res.results[k]["cs"], dtype=np.float64).reshape(-1)

    num_xy, G, diag_self = _positive_terms(
        x.astype(np.float64), yf.astype(np.float64)
    )
    return _finish(rs_seg, num_xy, G, diag_self)


# revision 28
# speedup vs baseline: 1.0187x; 1.0187x over previous
"""Trainium2 Bass kernel for nn_ContrastiveLoss_76501957477132.

Math (see module docstring history): with T=0.3, n=512 tracks, Q=8,
M=8192, D=128: per track i the loss needs per-track sums of
exp(x@x.T/T) (all pairs) and exp(x@yf.T/T) (all y cols), plus positive
terms computed exactly on host in float64.

v2 architecture (per core; 8 cores via band pairing k / 15-k):
  Inputs are fp8e4 (quantization error averages out in the 100K-element
  denominator sums; validated 3.6e-5 end-to-end).  Work unit = one
  [128 x 512] matmul tile; 33 units per group x 4 groups = 132 units.
  PSUM: ring of 3 slots x [128, 2, 512] f32 (6 banks) + 1 colsum
  accumulation bank.  Two consumer streams alternate on the ring:
   - ACT slots (8/group = off pairs + (off14, diagA)): ScalarE exp ->
     fp8e4 strip (feeds DoubleRow ones-matmul colsums for the E_xx
     mirror) with the rowsum free via accum_out.
   - DVE slots (34 flat = 4 diagB + 64 transposed-xy units): Schraudolph
     bit-trick exp straight to fp8e4 BITS: u8 = rne(z*S8 + B8) IS the
     fp8e4 encoding of exp(z/T).  These tiles are reduced purely by
     DoubleRow ones-matmul colsums (xy is computed transposed,
     [128 y x 512 x], so the per-x sums ARE colsums), so DVE does a
     single 1x PSUM pass and needs no second reduction pass anywhere.
  Colsum matmuls are emitted in deferred batches of 6 so the ones
  weights stay loaded and the PE never stalls on a strip.  Host folds
  rowsum/colsum partials by row residue (mod 512) and finishes the
  log/mean in float64.
"""

import numpy as np
import ml_dtypes

M, D, N_TRACKS, Q = 8192, 128, 512, 8
NQ = N_TRACKS * Q  # 4096
TEMP = 0.3
N_CORES = 8
N_BANDS = M // N_TRACKS          # 16 bands of 512 rows
GROUPS = 4                       # 128-row groups per band
OFF_UNITS = 15                   # off-diagonal xx units per group
N_RHS = 25                       # 15 off + diagA + diagB + 8 yf blocks
A_SLOTS = 8                      # ACT slots per group (2 units each)
D_SLOTS = 34                     # DVE slots per core (2 units each, flat)
T_TILES = 32                     # transposed-xy y-tiles per band (all of xy)
N_ACC = A_SLOTS                  # rowsum slots per group

# Schraudolph constants: fp8e4 bits of exp(z/TEMP)
SCH8_S = 8.0 / (TEMP * np.log(2.0))
SCH8_B = 7.0 * 8.0 - 0.45

_CACHED = {}


def _build_module():
    import concourse.bacc as bacc
    import concourse.tile as tile
    import concourse.mybir as mybir

    nc = bacc.Bacc(None, target_bir_lowering=False)
    bf16 = mybir.dt.bfloat16
    f32 = mybir.dt.float32
    f8 = mybir.dt.float8e4
    u8 = mybir.dt.uint8
    ALU = mybir.AluOpType

    lhsT_d = nc.dram_tensor("lhsT", [128, GROUPS, 2, 128], f8, kind="ExternalInput")
    lhsT_off_d = nc.dram_tensor(
        "lhsT_off", [128, GROUPS, OFF_UNITS, 128], f8, kind="ExternalInput"
    )
    rhs_d = nc.dram_tensor("rhs", [128, N_RHS, 512], f8, kind="ExternalInput")
    acc_d = nc.dram_tensor("acc", [128, GROUPS, N_ACC], f32, kind="ExternalOutput")
    cs_d = nc.dram_tensor("cs", [1, 512], f32, kind="ExternalOutput")

    # ACT slot tables per group: (stat, rhs_idx); stat: 0=A 1=B 2+u=off u
    a_slots = []
    for j in range(7):
        a_slots.append(((2 + 2 * j, 2 * j), (2 + 2 * j + 1, 2 * j + 1)))
    a_slots.append(((2 + 14, 14), (0, 15)))          # (off14, diagA)
    assert len(a_slots) == A_SLOTS
    # DVE slot table, flat over the core: colsum-only tiles.
    # ("diag", s): diagB of group s.  ("T", band_r, t): transposed xy,
    # stationary = y-tile t (128 rows of yf), moving = x band block.
    d_units = [("diag", s) for s in range(GROUPS)]
    d_units += [("T", 16, t) for t in range(T_TILES)]
    d_units += [("T", 15, t) for t in range(T_TILES)]
    d_slots = [tuple(d_units[2 * i : 2 * i + 2]) for i in range(D_SLOTS)]
    assert len(d_units) == 2 * D_SLOTS

    with tile.TileContext(nc) as tc:
        with (
            tc.tile_pool(name="consts", bufs=1) as consts,
            tc.tile_pool(name="accp", bufs=1) as accp,
            tc.tile_pool(name="ostrip", bufs=2) as ostrip_pool,
            tc.tile_pool(name="dstrip", bufs=12) as dstrip_pool,
            tc.tile_pool(name="psum", bufs=3, space="PSUM") as psum_pool,
            tc.tile_pool(name="cspsum", bufs=1, space="PSUM") as cs_pool,
        ):
            # ---- warmups: ACT exp table load + PE dummy matmuls ----
            warm = consts.tile([128, 1], f32, tag="warm")
            nc.vector.memset(warm[:], 0.0)
            warm_o = consts.tile([128, 1], bf16, tag="warm_o")
            nc.scalar.activation(
                out=warm_o[:], in_=warm[:],
                func=mybir.ActivationFunctionType.Exp, scale=1.0 / TEMP,
            )
            ones8 = consts.tile([128, 2, 16], f8, tag="ones8")
            nc.vector.memset(ones8[:], 1.0)

            # ---- input DMAs, roughly in first-consumption order ----
            lhsT = consts.tile([128, GROUPS, 2, 128], f8, tag="lhsT")
            lhsT_off = consts.tile([128, GROUPS, OFF_UNITS, 128], f8, tag="lhsT_off")
            rhs = consts.tile([128, N_RHS, 512], f8, tag="rhs")
            # D-slots are front-loaded in the schedule and need only lhsT +
            # rhs[16:18]; land those first (split across rings/engines), then
            # the first A-slot inputs, then the rest of the stream.
            nc.sync.dma_start(lhsT[0:64], lhsT_d[0:64])
            nc.scalar.dma_start(lhsT[64:128], lhsT_d[64:128])
            nc.sync.dma_start(rhs[0:64, 16:18], rhs_d[0:64, 16:18])
            nc.scalar.dma_start(rhs[64:128, 16:18], rhs_d[64:128, 16:18])
            nc.sync.dma_start(rhs[0:64, 0:2], rhs_d[0:64, 0:2])
            nc.scalar.dma_start(rhs[64:128, 0:2], rhs_d[64:128, 0:2])
            nc.sync.dma_start(lhsT_off[0:64, 0], lhsT_off_d[0:64, 0])
            nc.scalar.dma_start(lhsT_off[64:128, 0], lhsT_off_d[64:128, 0])
            nc.sync.dma_start(rhs[:, 18:21], rhs_d[:, 18:21])
            nc.scalar.dma_start(rhs[:, 2:4], rhs_d[:, 2:4])
            nc.sync.dma_start(rhs[:, 4:6], rhs_d[:, 4:6])
            nc.scalar.dma_start(rhs[:, 15:16], rhs_d[:, 15:16])
            nc.sync.dma_start(rhs[:, 6:8], rhs_d[:, 6:8])
            nc.scalar.dma_start(rhs[:, 21:25], rhs_d[:, 21:25])
            nc.sync.dma_start(rhs[:, 8:12], rhs_d[:, 8:12])
            nc.sync.dma_start(rhs[:, 12:15], rhs_d[:, 12:15])
            nc.sync.dma_start(lhsT_off[:, 1], lhsT_off_d[:, 1])
            nc.sync.dma_start(lhsT_off[:, 2], lhsT_off_d[:, 2])
            nc.sync.dma_start(lhsT_off[:, 3], lhsT_off_d[:, 3])

            acc_sb = accp.tile([128, GROUPS, N_ACC], f32)
            cs_ps = cs_pool.tile([1, 512], f32, name="cs_ps")

            n_cs = GROUPS * 8 + D_SLOTS  # per-group off 7 pairs+1 single; D pairs
            cs_done = 0
            pending = []  # deferred colsum matmuls: (rhs_ap, n_tiles)

            def flush_pending(k=2):
                nonlocal cs_done
                for ap, n in pending[:k]:
                    cs_done += 1
                    if n == 2:
                        nc.tensor.matmul(
                            cs_ps[:], ones8[:, :, 0:1], ap,
                            start=(cs_done == 1), stop=(cs_done == n_cs),
                            perf_mode=mybir.MatmulPerfMode.DoubleRow,
                            skip_group_check=True,
                        )
                    else:
                        nc.tensor.matmul(
                            cs_ps[:], ones8[:, 0, 0:1], ap,
                            start=(cs_done == 1), stop=(cs_done == n_cs),
                            skip_group_check=True,
                        )
                del pending[:k]

            def stat_ap(s, code):
                if code >= 2:
                    return lhsT_off[:, s, code - 2, :]
                return lhsT[:, s, code, :]

            # interleave the 4x9 ACT slots with the 30 DVE slots (Bresenham)
            sched = [("D", i) for i in range(4)]  # ramp filler: tiny inputs
            na, nd = GROUPS * A_SLOTS, D_SLOTS - 4
            ia = err = 0
            id_ = 4
            for _ in range(na + nd):
                if (err >= 0 and ia < na) or id_ >= D_SLOTS:
                    sched.append(("A", ia)); ia += 1; err -= nd
                else:
                    sched.append(("D", id_)); id_ += 1; err += na
            ostrips = {}
            for kind, idx in sched:
                ps = psum_pool.tile([128, 2, 512], f32, name="ring")
                if kind == "A":
                    s, j = divmod(idx, A_SLOTS)
                    if j == 0:
                        ostrips[s] = ostrip_pool.tile(
                            [128, A_SLOTS, 2, 512], f8, name=f"ostrip{s % 2}"
                        )
                    ostrip = ostrips[s]
                    for e, (code, r) in enumerate(a_slots[j]):
                        nc.tensor.matmul(
                            ps[:, e, :], stat_ap(s, code), rhs[:, r, :],
                            start=True, stop=True,
                        )
                    nc.scalar.activation(
                        out=ostrip[:, j], in_=ps[:],
                        func=mybir.ActivationFunctionType.Exp,
                        scale=1.0 / TEMP,
                        accum_out=acc_sb[:, s, j : j + 1],
                    )
                    if j < 7:
                        pending.append((ostrip[:, j], 2))
                    else:
                        pending.append((ostrip[:, j, 0], 1))
                        # [128, 8] f32 = 128 one-line descriptors: ~5us on a
                        # single ring.  Split by partitions so the final
                        # group's results drain in ~1.5us instead.
                        if s < GROUPS - 1:
                            nc.sync.dma_start(acc_d[:, s], acc_sb[:, s])
                        else:
                            nc.sync.dma_start(acc_d[0:64, s], acc_sb[0:64, s])
                            nc.sync.dma_start(acc_d[64:128, s], acc_sb[64:128, s])
                else:
                    for e, unit in enumerate(d_slots[idx]):
                        if unit[0] == "diag":
                            st, mv = lhsT[:, unit[1], 1, :], rhs[:, 16, :]
                        else:
                            _, band_r, t = unit
                            o = 128 * (t % 4)
                            st = rhs[:, 17 + t // 4, o : o + 128]
                            mv = rhs[:, band_r, :]
                        nc.tensor.matmul(ps[:, e, :], st, mv, start=True, stop=True)
                    dstrip = dstrip_pool.tile([128, 2, 512], u8, name="dstrip")
                    nc.vector.tensor_scalar(
                        dstrip[:], ps[:], SCH8_S, SCH8_B, ALU.mult, ALU.add
                    )
                    pending.append((dstrip[:].bitcast(f8), 2))
                if len(pending) >= 6:
                    flush_pending(6)
            flush_pending(len(pending))
            cs_sb = accp.tile([1, 512], f32, tag="cs_sb")
            nc.vector.tensor_copy(cs_sb[:], cs_ps[:])
            nc.sync.dma_start(cs_d[:], cs_sb[:])
    nc.compile()
    return nc


def _get_module():
    if "nc" not in _CACHED:
        _CACHED["nc"] = _build_module()
    return _CACHED["nc"]


def _positive_terms(x64, yf64):
    """num_xy, G, diag_self as float64 [512] vectors (exact math)."""
    xs = x64.reshape(N_BANDS, N_TRACKS, D)
    yfs = yf64.reshape(NQ // N_TRACKS, N_TRACKS, D)
    dxx = np.einsum("rid,cid->rci", xs, xs)
    dxy = np.einsum("rid,qid->rqi", xs, yfs)
    G = np.exp(dxx / TEMP).sum(axis=(0, 1))
    diag_self = np.exp(np.einsum("rid,rid->ri", xs, xs) / TEMP).sum(axis=0)
    num_xy = np.exp(dxy / TEMP).sum(axis=(0, 1))
    return num_xy, G, diag_self


def _finish(rs_seg, num_xy, G, diag_self):
    num = num_xy + (G - diag_self) / 2.0
    den = rs_seg - num_xy - G
    loss = np.mean(-np.log(num / (den + num))) / Q
    return np.asarray(loss, dtype=np.float32)


def _numpy_fallback(x, track_idxs, y):
    """Exact general-track reference in numpy (safety net only)."""
    x64 = x.astype(np.float64)
    yf64 = y.reshape(NQ, D).astype(np.float64)
    t = track_idxs.astype(np.int64)
    y_idxs = np.tile(np.arange(N_TRACKS, dtype=np.int64), Q)
    E_xy = np.exp(x64 @ yf64.T / TEMP)
    Sx = np.zeros((N_TRACKS, NQ))
    np.add.at(Sx, t, E_xy)
    Py = (y_idxs[:, None] == np.arange(N_TRACKS)[None, :]).astype(np.float64)
    num_xy = np.einsum("ik,ki->i", Sx, Py)
    den_xy = Sx.sum(axis=1) - num_xy
    E_xx = np.exp(x64 @ x64.T / TEMP)
    Sxx = np.zeros((N_TRACKS, M))
    np.add.at(Sxx, t, E_xx)
    Px = (t[:, None] == np.arange(N_TRACKS)[None, :]).astype(np.float64)
    G_diag = np.einsum("im,mi->i", Sxx, Px)
    diag_self = np.zeros(N_TRACKS)
    np.add.at(diag_self, t, np.diagonal(E_xx))
    num_xx = (G_diag - diag_self) / 2.0
    den_xx = Sxx.sum(axis=1) - G_diag
    num = num_xy + num_xx
    den = den_xy + den_xx
    loss = np.mean(-np.log(num / (den + num))) / Q
    return np.asarray(loss, dtype=np.float32)


def kernel(x, track_idxs, y):
    x = np.asarray(x, dtype=np.float32)
    y = np.asarray(y, dtype=np.float32)
    track_idxs = np.asarray(track_idxs)

    expected_tracks = np.arange(M, dtype=np.int64) % N_TRACKS
    if (
        x.shape != (M, D)
        or y.shape != (N_TRACKS, Q, D)
        or not np.array_equal(track_idxs.astype(np.int64), expected_tracks)
    ):
        return _numpy_fallback(x, track_idxs, y)

    from concourse.bass_utils import run_bass_kernel_spmd

    f8np = ml_dtypes.float8_e4m3
    yf = np.ascontiguousarray(y.reshape(NQ, D))
    xT = np.ascontiguousarray(x.T).astype(f8np)     # [128, 8192]
    yfT = np.ascontiguousarray(yf.T).astype(f8np)   # [128, 4096]
    xT_blocks = xT.reshape(128, N_BANDS, 512)
    yfT_blocks = yfT.reshape(128, Q, 512)

    def subtile(band, s):
        t = 4 * band + s
        return xT[:, 128 * t : 128 * (t + 1)]

    in_maps = []
    for k in range(N_CORES):
        A, B = k, (N_BANDS - 1) - k
        off_cols = list(range(A + 1, N_BANDS)) + list(range(B + 1, N_BANDS))
        assert len(off_cols) == OFF_UNITS
        rhs = np.empty((128, N_RHS, 512), dtype=f8np)
        for u, c in enumerate(off_cols):
            rhs[:, u] = xT_blocks[:, c]
        rhs[:, 15] = xT_blocks[:, A]
        rhs[:, 16] = xT_blocks[:, B]
        for q in range(Q):
            rhs[:, 17 + q] = yfT_blocks[:, q]

        lhsT = np.empty((128, GROUPS, 2, 128), dtype=f8np)
        lhsT_off = np.empty((128, GROUPS, OFF_UNITS, 128), dtype=f8np)
        for s in range(GROUPS):
            lhsT[:, s, 0, :] = subtile(A, s)
            lhsT[:, s, 1, :] = subtile(B, s)
            for u in range(OFF_UNITS):
                band = A if u < (N_BANDS - 1) - k else B
                lhsT_off[:, s, u, :] = subtile(band, s)
        in_maps.append(
            {
                "lhsT": np.ascontiguousarray(lhsT),
                "lhsT_off": np.ascontiguousarray(lhsT_off),
                "rhs": np.ascontiguousarray(rhs),
            }
        )

    nc = _get_module()
    res = run_bass_kernel_spmd(nc, in_maps, core_ids=list(range(N_CORES)))
    _CACHED["last_res"] = res

    # Fold partials by row residue (mod 512): group s covers residues
    # 128s + p; colsums fold by in-block column position directly.
    rs_seg = np.zeros(N_TRACKS, dtype=np.float64)
    for k in range(N_CORES):
        acc = np.asarray(res.results[k]["acc"], dtype=np.float64)  # [128, 4, 11]
        per_group = acc.sum(axis=2)                                # [128, 4]
        rs_seg += per_group.T.reshape(N_TRACKS)                    # i = 128*s + p
        rs_seg += np.asarray(res.results[k]["cs"], dtype=np.float64).reshape(-1)

    num_xy, G, diag_self = _positive_terms(
        x.astype(np.float64), yf.astype(np.float64)
    )
    return _finish(rs_seg, num_xy, G, diag_self)
